# revision 1
# baseline (speedup 1.0000x reference)
"""Trainium2 Bass kernel for a 2-layer GATv2 + GraphNorm node classifier.

Strategy (8 NeuronCores, SPMD single NEFF):
  - Nodes are sharded contiguously: core k owns nodes [k*6250, (k+1)*6250).
  - Host (index-only preprocessing): add self loops, route each edge to the
    owner of its dst, sort by dst, group into 128-node blocks, pad each block's
    edge list to a whole number of 128-edge tiles (schedule shared by all
    cores so one program serves all), remap src to a padded table index,
    pre-transpose x.
  - Device per core: build the full xl=x@Wl+bl gather table (replicated),
    local xr blocks; per 128-edge tile: one-hot(dst) matrix via is_equal,
    TensorE matmuls for xr broadcast + attention-dot + softmax-weighted
    scatter-add accumulated in PSUM per 128-node block.  GraphNorm is folded
    into the next layer's weights (stats via matmul + AllReduce[64,2]);
    layer-2 gather table built after AllGather of h1 (transposed layout).
  - Softmax max-subtraction is skipped (|e| bounded ~<10, exp is safe in f32).
"""

import numpy as np

import concourse.bacc as bacc
import concourse.bass as bass
import concourse.mybir as mybir
import concourse.tile as tile
from concourse.masks import make_identity

F32 = mybir.dt.float32
I32 = mybir.dt.int32
AF = mybir.ActivationFunctionType
OP = mybir.AluOpType

P = 128


class Cfg:
    def __init__(self, n_nodes, n_cores=8):
        assert n_nodes % n_cores == 0
        self.N = n_nodes
        self.NC = n_cores
        self.NPC = n_nodes // n_cores          # real nodes per core
        self.BLOCKS = (self.NPC + P - 1) // P  # 128-node blocks per core
        self.NPADC = self.BLOCKS * P           # padded nodes per core
        self.NPAD_ALL = self.NC * self.NPADC   # padded table rows
        self.DIN = 128
        self.HC = 128                          # H*C
        self.C = 64
        self.NCLS = 4
        self.EPS = 1e-5


def _preprocess(cfg, x, edge_index):
    """Host-side index preprocessing + input staging. Returns (T_list, in_maps_extra)."""
    N, NC, NPC, BLOCKS, NPADC = cfg.N, cfg.NC, cfg.NPC, cfg.BLOCKS, cfg.NPADC
    E = edge_index.shape[1]
    src = np.concatenate([edge_index[0].astype(np.int64), np.arange(N, dtype=np.int64)])
    dst = np.concatenate([edge_index[1].astype(np.int64), np.arange(N, dtype=np.int64)])

    core = dst // NPC
    dloc = dst - core * NPC
    blk = dloc // P
    dstl = dloc - blk * P                      # within-block dst index [0,128)
    gb = core * BLOCKS + blk                   # global (core, block) id

    # per-(core,block) counts -> shared tile schedule
    cnt = np.bincount(gb, minlength=NC * BLOCKS).reshape(NC, BLOCKS)
    T_list = np.maximum(1, (cnt.max(axis=0) + P - 1) // P).astype(np.int64)  # [BLOCKS]
    T_total = int(T_list.sum())
    offs = np.concatenate([[0], np.cumsum(T_list)])  # tile offset per block

    srcr = (src // NPC) * NPADC + (src % NPC)  # remapped src (padded table row)

    esrcT = np.zeros((NC, P, T_total), dtype=np.int32)
    edstlT = np.full((NC, P, T_total), -1.0, dtype=np.float32)

    order = np.lexsort((dstl, gb))  # sort edges by (core, block) then dstl (any in-block order ok)
    gb_s, dstl_s, srcr_s = gb[order], dstl[order], srcr[order]
    # slot position of each edge within its (core, block) group
    pos_in_group = np.arange(len(gb_s)) - np.searchsorted(gb_s, gb_s, side="left")
    core_s = gb_s // BLOCKS
    blk_s = gb_s % BLOCKS
    slot = offs[blk_s] * P + pos_in_group      # flat slot inside this core's edge array
    tile_i = slot // P
    part_i = slot % P
    esrcT[core_s, part_i, tile_i] = srcr_s
    edstlT[core_s, part_i, tile_i] = dstl_s.astype(np.float32)

    # transposed, padded x
    xT = np.zeros((cfg.DIN, cfg.NPAD_ALL), dtype=np.float32)
    xsrc = np.ascontiguousarray(x.T)  # [DIN, N]
    for k in range(NC):
        xT[:, k * NPADC : k * NPADC + NPC] = xsrc[:, k * NPC : (k + 1) * NPC]

    per_core = []
    for k in range(NC):
        per_core.append({
            "xT": np.ascontiguousarray(xT),
            "xTl": np.ascontiguousarray(xT[:, k * NPADC : (k + 1) * NPADC]),
            "esrcT": np.ascontiguousarray(esrcT[k]),
            "edstlT": np.ascontiguousarray(edstlT[k]),
        })
    return [int(t) for t in T_list], per_core


def _build(cfg, T_list):
    """Build + compile the SPMD program. Returns nc."""
    NC, BLOCKS, NPADC, NPAD_ALL = cfg.NC, cfg.BLOCKS, cfg.NPADC, cfg.NPAD_ALL
    NPC, HC, C, NCLS = cfg.NPC, cfg.HC, cfg.C, cfg.NCLS
    T_total = sum(T_list)
    NT = NC * BLOCKS
    rg = [list(range(NC))]
    LAST = NPC - (BLOCKS - 1) * P  # real rows in last block

    nc = bacc.Bacc("TRN2", target_bir_lowering=False, debug=False,
                   enable_asserts=False, num_devices=NC)

    # ---------------- IO ----------------
    xT_d = nc.dram_tensor("xT", [128, NPAD_ALL], F32, kind="ExternalInput")
    xTl_d = nc.dram_tensor("xTl", [128, NPADC], F32, kind="ExternalInput")
    esrcT_d = nc.dram_tensor("esrcT", [P, T_total], I32, kind="ExternalInput")
    edstlT_d = nc.dram_tensor("edstlT", [P, T_total], F32, kind="ExternalInput")
    w = {}
    for li, din in ((1, 128), (2, 64)):
        w[f"Wl{li}"] = nc.dram_tensor(f"Wl{li}", [din, HC], F32, kind="ExternalInput")
        w[f"Wr{li}"] = nc.dram_tensor(f"Wr{li}", [din, HC], F32, kind="ExternalInput")
        w[f"bl{li}"] = nc.dram_tensor(f"bl{li}", [HC], F32, kind="ExternalInput")
        w[f"br{li}"] = nc.dram_tensor(f"br{li}", [HC], F32, kind="ExternalInput")
        w[f"att{li}"] = nc.dram_tensor(f"att{li}", [2, C], F32, kind="ExternalInput")
        w[f"bias{li}"] = nc.dram_tensor(f"bias{li}", [C], F32, kind="ExternalInput")
        w[f"gng{li}"] = nc.dram_tensor(f"gng{li}", [C], F32, kind="ExternalInput")
        w[f"gnb{li}"] = nc.dram_tensor(f"gnb{li}", [C], F32, kind="ExternalInput")
        w[f"gna{li}"] = nc.dram_tensor(f"gna{li}", [C], F32, kind="ExternalInput")
    W1_d = nc.dram_tensor("W1", [C, NCLS], F32, kind="ExternalInput")
    b1_d = nc.dram_tensor("b1", [NCLS], F32, kind="ExternalInput")
    out_d = nc.dram_tensor("out", [NPC, NCLS], F32, kind="ExternalOutput")
    import os as _os
    DBG = bool(int(_os.environ.get("GAT_DEBUG", "0")))
    if DBG:
        dbg_xl1 = nc.dram_tensor("dbg_xl1", [256, HC], F32, kind="ExternalOutput")
        dbg_h1T = nc.dram_tensor("dbg_h1T", [C, NPADC], F32, kind="ExternalOutput")
        dbg_st = nc.dram_tensor("dbg_st", [C, 2], F32, kind="ExternalOutput")
        dbg_xr1 = nc.dram_tensor("dbg_xr1", [P, HC], F32, kind="ExternalOutput")
        dbg_AB = nc.dram_tensor("dbg_AB", [C, 2], F32, kind="ExternalOutput")
        dbg_ag = nc.dram_tensor("dbg_ag", [C * NC, 128], F32, kind="ExternalOutput")
        dbg_xl2 = nc.dram_tensor("dbg_xl2", [256, HC], F32, kind="ExternalOutput")
        dbg_xr2 = nc.dram_tensor("dbg_xr2", [P, HC], F32, kind="ExternalOutput")
        dbg_h2T = nc.dram_tensor("dbg_h2T", [C, NPADC], F32, kind="ExternalOutput")

    # ---------------- internal DRAM ----------------
    xl1_t = nc.dram_tensor("xl1_t", [NPAD_ALL, HC], F32, kind="Internal")
    xl2_t = nc.dram_tensor("xl2_t", [NPAD_ALL, HC], F32, kind="Internal")
    h1T_dr = nc.dram_tensor("h1T_dr", [C, NPADC], F32, kind="Internal")
    h1T_ag = nc.dram_tensor("h1T_ag", [C * NC, NPADC], F32, kind="Internal", addr_space="Shared")
    st_l = [nc.dram_tensor(f"st{li}_l", [C, 2], F32, kind="Internal") for li in (1, 2)]
    st_g = [nc.dram_tensor(f"st{li}_g", [C, 2], F32, kind="Internal", addr_space="Shared") for li in (1, 2)]

    with tile.TileContext(nc) as tc:
        import contextlib
        ctx = contextlib.ExitStack()
        with ctx:
            con = ctx.enter_context(tc.tile_pool(name="con", bufs=1))
            res = ctx.enter_context(tc.tile_pool(name="res", bufs=1))
            sb = ctx.enter_context(tc.tile_pool(name="sb", bufs=4))
            sgath = ctx.enter_context(tc.tile_pool(name="sgath", bufs=6))
            sidx = ctx.enter_context(tc.tile_pool(name="sidx", bufs=2))
            ps_t = ctx.enter_context(tc.tile_pool(name="ps_t", bufs=1, space="PSUM"))
            ps_b = ctx.enter_context(tc.tile_pool(name="ps_b", bufs=2, space="PSUM"))
            ps_e = ctx.enter_context(tc.tile_pool(name="ps_e", bufs=1, space="PSUM"))
            ps_pet = ctx.enter_context(tc.tile_pool(name="ps_pet", bufs=1, space="PSUM"))
            ps_acc = ctx.enter_context(tc.tile_pool(name="ps_acc", bufs=2, space="PSUM"))
            ps_st = ctx.enter_context(tc.tile_pool(name="ps_st", bufs=1, space="PSUM"))

            # ---------------- constants ----------------
            ident = con.tile([P, P], F32)
            make_identity(nc, ident[:])
            iota_i = con.tile([P, P], I32)
            nc.gpsimd.iota(iota_i[:], pattern=[[1, P]], base=0, channel_multiplier=0)
            iota_f = con.tile([P, P], F32)
            nc.vector.tensor_copy(iota_f[:], iota_i[:])
            ones_col = con.tile([P, 1], F32)
            nc.vector.memset(ones_col[:], 1.0)
            ones_row = con.tile([1, P], F32)
            nc.vector.memset(ones_row[:], 1.0)
            # mask column: 1.0 for real rows of the last node block, 0 for pad
            mask_col = con.tile([P, 1], F32)
            nc.vector.memset(mask_col[:], 1.0)
            if LAST < P:
                nc.gpsimd.affine_select(
                    out=mask_col[:], in_=mask_col[:], compare_op=OP.is_ge,
                    fill=0.0, base=LAST - 1, channel_multiplier=-1, pattern=[[0, 1]])

            def load_row(d, n):  # [n] dram -> [1, n] sbuf
                t = con.tile([1, n], F32, tag=f"row_{d.name}")
                nc.sync.dma_start(out=t[:], in_=d[None, :])
                return t

            def load_col(d, n):  # [n] dram -> [n, 1] sbuf
                t = con.tile([n, 1], F32, tag=f"col_{d.name}")
                nc.sync.dma_start(out=t[:], in_=d[:, None])
                return t

            def replicate_row(row_t, n, tag):  # [1,n] -> [P,n]
                pr = ps_b.tile([P, n], F32, space="PSUM", tag="ps_mm")
                nc.tensor.matmul(pr[:], lhsT=ones_row[:], rhs=row_t[:], start=True, stop=True)
                t = con.tile([P, n], F32, tag=tag)
                nc.scalar.copy(t[:], pr[:])
                return t

            Wl1_sb = con.tile([128, HC], F32)
            nc.sync.dma_start(out=Wl1_sb[:], in_=w["Wl1"][:])
            Wr1_sb = con.tile([128, HC], F32)
            nc.sync.dma_start(out=Wr1_sb[:], in_=w["Wr1"][:])
            Wl2_sb = con.tile([C, HC], F32)
            nc.sync.dma_start(out=Wl2_sb[:], in_=w["Wl2"][:])
            Wr2_sb = con.tile([C, HC], F32)
            nc.sync.dma_start(out=Wr2_sb[:], in_=w["Wr2"][:])
            W1_sb = con.tile([C, NCLS], F32)
            nc.sync.dma_start(out=W1_sb[:], in_=W1_d[:])
            b1_row = load_row(b1_d, NCLS)

            bl1_rep = replicate_row(load_row(w["bl1"], HC), HC, "bl1_rep")
            br1_rep = replicate_row(load_row(w["br1"], HC), HC, "br1_rep")
            bias_rep = [replicate_row(load_row(w[f"bias{li}"], C), C, f"bias{li}_rep") for li in (1, 2)]

            attm = []
            for li in (1, 2):
                t = con.tile([P, 2], F32, tag=f"attm{li}")
                nc.vector.memset(t[:], 0.0)
                nc.sync.dma_start(out=t[0:C, 0:1], in_=w[f"att{li}"][0, :][:, None])
                nc.sync.dma_start(out=t[C:2 * C, 1:2], in_=w[f"att{li}"][1, :][:, None])
                attm.append(t)

            # ---------------- layer-1 tables ----------------
            xr1_res = res.tile([P, BLOCKS, HC], F32, tag="xr1res")
            for j in range(NT):
                xt = sb.tile([128, P], F32, tag="xt")
                nc.sync.dma_start(out=xt[:], in_=xT_d[:, j * P:(j + 1) * P])
                pm = ps_b.tile([P, HC], F32, space="PSUM", tag="ps_mm")
                nc.tensor.matmul(pm[:], lhsT=xt[:], rhs=Wl1_sb[:], start=True, stop=True)
                xlt = sb.tile([P, HC], F32, tag="xlt")
                nc.vector.tensor_add(xlt[:], pm[:], bl1_rep[:])
                nc.sync.dma_start(out=xl1_t[j * P:(j + 1) * P, :], in_=xlt[:])
            for b in range(BLOCKS):
                xt = sb.tile([128, P], F32, tag="xt")
                nc.sync.dma_start(out=xt[:], in_=xTl_d[:, b * P:(b + 1) * P])
                pm = ps_b.tile([P, HC], F32, space="PSUM", tag="ps_mm")
                nc.tensor.matmul(pm[:], lhsT=xt[:], rhs=Wr1_sb[:], start=True, stop=True)
                nc.vector.tensor_add(xr1_res[:, b, :], pm[:], br1_rep[:])

            # ---------------- edge phase (shared for both layers) ----------------
            h1T_res = res.tile([C, NPADC], F32, tag="h1T")
            h2T_res = res.tile([C, NPADC], F32, tag="h2T")

            def edge_layer(li, table, xr_res, hT_res, b_rep):
                pstats = ps_st.tile([C, C + 1], F32, space="PSUM", tag="ps_stats")
                for b in range(BLOCKS):
                    Tb = T_list[b]
                    c0 = sum(T_list[:b])
                    srcg = sidx.tile([P, Tb], I32, tag="srcg")
                    nc.sync.dma_start(out=srcg[:], in_=esrcT_d[:, c0:c0 + Tb])
                    dstg = sidx.tile([P, Tb], F32, tag="dstg")
                    nc.sync.dma_start(out=dstg[:], in_=edstlT_d[:, c0:c0 + Tb])
                    acc = ps_acc.tile([P, HC + 2], F32, space="PSUM", tag="ps_acc")
                    for t in range(Tb):
                        oh = sb.tile([P, P], F32, tag="oh")
                        nc.vector.tensor_tensor(out=oh[:], in0=iota_f[:],
                                                in1=dstg[:, t:t + 1].to_broadcast([P, P]),
                                                op=OP.is_equal)
                        pt = ps_t.tile([P, P], F32, space="PSUM", tag="ps_tr")
                        nc.tensor.transpose(pt[:], oh[:], ident[:])
                        ohT = sb.tile([P, P], F32, tag="ohT")
                        nc.vector.tensor_copy(ohT[:], pt[:])
                        xls = sgath.tile([P, HC], F32, tag="xls")
                        nc.gpsimd.indirect_dma_start(
                            out=xls[:], out_offset=None, in_=table[:],
                            in_offset=bass.IndirectOffsetOnAxis(ap=srcg[:, t:t + 1], axis=0))
                        pb = ps_b.tile([P, P], F32, space="PSUM", tag="ps_mm")
                        nc.tensor.matmul(pb[:], lhsT=xls[:], rhs=ident[:], start=True, stop=False)
                        nc.tensor.matmul(pb[:], lhsT=xr_res[:, b, :], rhs=ohT[:], start=False, stop=True)
                        s02 = sb.tile([P, P], F32, tag="s02")
                        nc.scalar.activation(s02[:], pb[:], AF.Copy, bias=0.0, scale=0.2)
                        r08 = sb.tile([P, P], F32, tag="r08")
                        nc.scalar.activation(r08[:], pb[:], AF.Relu, bias=0.0, scale=0.8)
                        pe = ps_e.tile([2, P], F32, space="PSUM", tag="ps_e")
                        nc.tensor.matmul(pe[:], lhsT=attm[li - 1][:], rhs=s02[:], start=True, stop=False)
                        nc.tensor.matmul(pe[:], lhsT=attm[li - 1][:], rhs=r08[:], start=False, stop=True)
                        eeT = sb.tile([2, P], F32, tag="eeT")
                        nc.scalar.activation(eeT[:], pe[:], AF.Exp)
                        pet = ps_pet.tile([P, 2], F32, space="PSUM", tag="ps_pet")
                        nc.tensor.transpose(pet[:], eeT[:], ident[0:2, 0:2])
                        pay = sb.tile([P, HC + 2], F32, tag="pay")
                        nc.vector.tensor_copy(pay[:, HC:HC + 2], pet[:])
                        nc.vector.tensor_scalar_mul(pay[:, 0:C], xls[:, 0:C], pay[:, HC:HC + 1])
                        nc.vector.tensor_scalar_mul(pay[:, C:HC], xls[:, C:HC], pay[:, HC + 1:HC + 2])
                        nc.tensor.matmul(acc[:], lhsT=oh[:], rhs=pay[:], start=(t == 0), stop=(t == Tb - 1))
                    # ---- drain block b ----
                    last = b == BLOCKS - 1
                    # bias keeps pad-row denominators finite (0 -> 1e-20)
                    d2 = sb.tile([P, 2], F32, tag="d2")
                    nc.scalar.activation(d2[:], acc[:, HC:HC + 2], AF.Copy, bias=1e-20, scale=2.0)
                    rec = sb.tile([P, 2], F32, tag="rec")
                    nc.vector.reciprocal(rec[:], d2[:])
                    t0 = sb.tile([P, C], F32, tag="t0")
                    nc.vector.tensor_scalar_mul(t0[:], acc[:, 0:C], rec[:, 0:1])
                    t1 = sb.tile([P, C], F32, tag="t1")
                    nc.vector.tensor_scalar_mul(t1[:], acc[:, C:HC], rec[:, 1:2])
                    hs = sb.tile([P, C + 1], F32, tag="hs")
                    nc.vector.memset(hs[:, C:C + 1], 1.0)
                    nc.vector.tensor_add(hs[:, 0:C], t0[:], t1[:])
                    hb = hs[:, 0:C]
                    nc.vector.tensor_add(hb, hb, b_rep[:])
                    if last and LAST < P:
                        nc.vector.tensor_scalar_mul(hs[:], hs[:], mask_col[:, 0:1])
                    nc.tensor.matmul(pstats[:], lhsT=hb, rhs=hs[:], start=(b == 0), stop=(b == BLOCKS - 1))
                    pht = ps_t.tile([C, P], F32, space="PSUM", tag="ps_tr")
                    nc.tensor.transpose(pht[:], hb, ident[:])
                    nc.scalar.copy(hT_res[:, b * P:(b + 1) * P], pht[:])
                # ---- stats finalize + AllReduce ----
                trash = sb.tile([C, C], F32, tag="trash")
                st2 = sb.tile([C, 2], F32, tag="st2")
                nc.vector.tensor_mul(trash[:], pstats[:, 0:C], ident[0:C, 0:C])
                nc.vector.tensor_reduce(st2[:, 1:2], trash[:], axis=mybir.AxisListType.X, op=OP.add)
                nc.vector.tensor_copy(st2[:, 0:1], pstats[:, C:C + 1])
                nc.sync.dma_start(out=st_l[li - 1][:], in_=st2[:])
                nc.gpsimd.collective_compute(
                    "AllReduce", OP.add, replica_groups=rg,
                    ins=[st_l[li - 1][:]], outs=[st_g[li - 1][:]])
                stg = sb.tile([C, 2], F32, tag="stg")
                nc.sync.dma_start(out=stg[:], in_=st_g[li - 1][:])
                # A = gng * rsqrt(var+eps); B = gnb - A*a*mean
                a_col = load_col(w[f"gna{li}"], C)
                g_col = load_col(w[f"gng{li}"], C)
                bta_col = load_col(w[f"gnb{li}"], C)
                mean = sb.tile([C, 1], F32, tag="gn_m")
                nc.scalar.activation(mean[:], stg[:, 0:1], AF.Copy, bias=0.0, scale=1.0 / cfg.N)
                msq = sb.tile([C, 1], F32, tag="gn_m2")
                nc.scalar.square(msq[:], mean[:])
                qn = sb.tile([C, 1], F32, tag="gn_qn")
                nc.scalar.activation(qn[:], stg[:, 1:2], AF.Copy, bias=0.0, scale=1.0 / cfg.N)
                a2 = sb.tile([C, 1], F32, tag="gn_a2")
                nc.vector.tensor_mul(a2[:], a_col[:], a_col[:])
                twoa = sb.tile([C, 1], F32, tag="gn_2a")
                nc.scalar.activation(twoa[:], a_col[:], AF.Copy, bias=0.0, scale=2.0)
                coef = sb.tile([C, 1], F32, tag="gn_cf")
                nc.vector.tensor_sub(coef[:], twoa[:], a2[:])
                cm = sb.tile([C, 1], F32, tag="gn_cm")
                nc.vector.tensor_mul(cm[:], coef[:], msq[:])
                var = sb.tile([C, 1], F32, tag="gn_var")
                nc.vector.tensor_sub(var[:], qn[:], cm[:])
                vare = sb.tile([C, 1], F32, tag="gn_vare")
                nc.vector.tensor_scalar_add(vare[:], var[:], cfg.EPS)
                lnv = sb.tile([C, 1], F32, tag="gn_lnv")
                nc.scalar.activation(lnv[:], vare[:], AF.Ln)
                rs = sb.tile([C, 1], F32, tag="gn_rs")
                nc.scalar.activation(rs[:], lnv[:], AF.Exp, bias=0.0, scale=-0.5)
                A = sb.tile([C, 1], F32, tag="gn_A")
                nc.vector.tensor_mul(A[:], g_col[:], rs[:])
                t_ = sb.tile([C, 1], F32, tag="gn_t")
                nc.vector.tensor_mul(t_[:], A[:], a_col[:])
                t2_ = sb.tile([C, 1], F32, tag="gn_t2")
                nc.vector.tensor_mul(t2_[:], t_[:], mean[:])
                B = sb.tile([C, 1], F32, tag="gn_B")
                nc.vector.tensor_sub(B[:], bta_col[:], t2_[:])
                return A, B

            A1, B1 = edge_layer(1, xl1_t, xr1_res, h1T_res, bias_rep[0])

            if DBG:
                nc.sync.dma_start(out=dbg_xl1[:], in_=xl1_t[0:256, :])
                nc.sync.dma_start(out=dbg_h1T[:], in_=h1T_res[:])
                nc.sync.dma_start(out=dbg_st[:], in_=st_g[0][:])
                nc.sync.dma_start(out=dbg_xr1[:], in_=xr1_res[:, 3, :])

            # AllGather h1 (transposed layout)
            nc.sync.dma_start(out=h1T_dr[:], in_=h1T_res[:])
            nc.gpsimd.collective_compute(
                "AllGather", OP.bypass, replica_groups=rg,
                ins=[h1T_dr[:]], outs=[h1T_ag[:]])

            # folded layer-2 weights
            def fold(W_sb, b_d, A, B, ncols, tag):
                Wp = con.tile([C, ncols], F32, tag=f"W_{tag}")
                nc.vector.tensor_scalar_mul(Wp[:], W_sb[:], A[:])
                pbias = ps_b.tile([1, ncols], F32, space="PSUM", tag="ps_mm")
                nc.tensor.matmul(pbias[:], lhsT=B[:], rhs=W_sb[:], start=True, stop=True)
                brow = con.tile([1, ncols], F32, tag=f"brow_{tag}")
                nc.vector.tensor_add(brow[:], pbias[:], load_row(b_d, ncols)[:])
                rep = replicate_row(brow, ncols, f"brep_{tag}")
                return Wp, rep

            Wl2p, bl2p_rep = fold(Wl2_sb, w["bl2"], A1, B1, HC, "l2l")
            Wr2p, br2p_rep = fold(Wr2_sb, w["br2"], A1, B1, HC, "l2r")

            # ---------------- layer-2 tables ----------------
            xr2_res = res.tile([P, BLOCKS, HC], F32, tag="xr2res")
            for j in range(NT):
                k, b = divmod(j, BLOCKS)
                ht = sb.tile([C, P], F32, tag="ht")
                nc.sync.dma_start(out=ht[:], in_=h1T_ag[k * C:(k + 1) * C, b * P:(b + 1) * P])
                pm = ps_b.tile([P, HC], F32, space="PSUM", tag="ps_mm")
                nc.tensor.matmul(pm[:], lhsT=ht[:], rhs=Wl2p[:], start=True, stop=True)
                xlt = sb.tile([P, HC], F32, tag="xlt")
                nc.vector.tensor_add(xlt[:], pm[:], bl2p_rep[:])
                nc.sync.dma_start(out=xl2_t[j * P:(j + 1) * P, :], in_=xlt[:])
            for b in range(BLOCKS):
                pm = ps_b.tile([P, HC], F32, space="PSUM", tag="ps_mm")
                nc.tensor.matmul(pm[:], lhsT=h1T_res[:, b * P:(b + 1) * P], rhs=Wr2p[:], start=True, stop=True)
                nc.vector.tensor_add(xr2_res[:, b, :], pm[:], br2p_rep[:])

            if DBG:
                nc.sync.dma_start(out=dbg_ag[:], in_=h1T_ag[:, 384:512])
                nc.sync.dma_start(out=dbg_xl2[:], in_=xl2_t[0:256, :])
                nc.sync.dma_start(out=dbg_xr2[:], in_=xr2_res[:, 3, :])
                ab = sb.tile([C, 2], F32, tag="dbgab")
                nc.vector.tensor_copy(ab[:, 0:1], A1[:])
                nc.vector.tensor_copy(ab[:, 1:2], B1[:])
                nc.sync.dma_start(out=dbg_AB[:], in_=ab[:])

            A2, B2 = edge_layer(2, xl2_t, xr2_res, h2T_res, bias_rep[1])

            if DBG:
                nc.sync.dma_start(out=dbg_h2T[:], in_=h2T_res[:])

            # ---------------- classifier + log_softmax ----------------
            W1p = con.tile([C, NCLS], F32, tag="W1p")
            nc.vector.tensor_scalar_mul(W1p[:], W1_sb[:], A2[:])
            pb1 = ps_b.tile([1, NCLS], F32, space="PSUM", tag="ps_mm")
            nc.tensor.matmul(pb1[:], lhsT=B2[:], rhs=W1_sb[:], start=True, stop=True)
            b1p = con.tile([1, NCLS], F32, tag="b1p")
            nc.vector.tensor_add(b1p[:], pb1[:], b1_row[:])
            b1p_rep = replicate_row(b1p, NCLS, "b1p_rep")

            for b in range(BLOCKS):
                pl = ps_acc.tile([P, NCLS], F32, space="PSUM", tag="ps_acc")
                nc.tensor.matmul(pl[:], lhsT=h2T_res[:, b * P:(b + 1) * P], rhs=W1p[:], start=True, stop=True)
                lg = sb.tile([P, NCLS], F32, tag="lg")
                nc.vector.tensor_add(lg[:], pl[:], b1p_rep[:])
                mx = sb.tile([P, 1], F32, tag="mx")
                nc.vector.tensor_reduce(mx[:], lg[:], axis=mybir.AxisListType.X, op=OP.max)
                lgm = sb.tile([P, NCLS], F32, tag="lgm")
                nc.vector.tensor_scalar(out=lgm[:], in0=lg[:], scalar1=mx[:, 0:1], scalar2=None, op0=OP.subtract)
                ex = sb.tile([P, NCLS], F32, tag="ex")
                nc.scalar.activation(ex[:], lgm[:], AF.Exp)
                sm = sb.tile([P, 1], F32, tag="sm")
                nc.vector.tensor_reduce(sm[:], ex[:], axis=mybir.AxisListType.X, op=OP.add)
                lns = sb.tile([P, 1], F32, tag="lns")
                nc.scalar.activation(lns[:], sm[:], AF.Ln)
                ot = sb.tile([P, NCLS], F32, tag="ot")
                nc.vector.tensor_scalar(out=ot[:], in0=lgm[:], scalar1=lns[:, 0:1], scalar2=None, op0=OP.subtract)
                rows = min(P, NPC - b * P)
                nc.sync.dma_start(out=out_d[b * P: b * P + rows, :], in_=ot[0:rows, :])

    nc.compile()
    return nc


_CACHE = {}


def _get_program(cfg, T_list):
    key = tuple(T_list)
    if key not in _CACHE:
        _CACHE[key] = _build(cfg, T_list)
    return _CACHE[key]


def _install_axon_ntff_shim():
    """Provide antenv.axon_hooks (missing on this image) so trace=True works
    under axon. Mirrors trn_agent_boot's ctypes hook against libaxon_pjrt.so."""
    import sys, types, ctypes, contextlib, glob as _glob
    try:
        import antenv.axon_hooks  # noqa
        return
    except ImportError:
        pass
    hook = None
    for so_path in (["/opt/axon/libaxon_pjrt.so"] + _glob.glob("/root/.axon_site/**/libaxon_pjrt.so", recursive=True)):
        try:
            lib = ctypes.CDLL(so_path)
        except OSError:
            continue
        if not hasattr(lib, "axon_start_nrt_profile"):
            continue
        lib.axon_start_nrt_profile.argtypes = [ctypes.POINTER(ctypes.c_int64), ctypes.c_size_t]
        lib.axon_start_nrt_profile.restype = ctypes.c_int64
        lib.axon_stop_nrt_profile.argtypes = [ctypes.c_char_p]
        lib.axon_stop_nrt_profile.restype = ctypes.c_int64

        @contextlib.contextmanager
        def _hook(output_dir, device_ids, _lib=lib):
            import jax
            jax.devices()
            if device_ids:
                ids = (ctypes.c_int64 * len(device_ids))(*device_ids)
                rc = _lib.axon_start_nrt_profile(ids, len(device_ids))
            else:
                rc = _lib.axon_start_nrt_profile(None, 0)
            if rc != 0:
                raise RuntimeError(f"axon_start_nrt_profile rc={rc}")
            try:
                yield
            finally:
                n = _lib.axon_stop_nrt_profile(str(output_dir).encode())
                print(f"ntff profile: {n} file(s) -> {output_dir}")

        hook = _hook
        break
    m = types.ModuleType("antenv.axon_hooks")
    m.get_axon_ntff_profile_hook = lambda: hook
    m.set_axon_ntff_profile_hook = lambda h: None
    sys.modules["antenv.axon_hooks"] = m
    try:
        import antenv
        antenv.axon_hooks = m
    except ImportError:
        pass
    # artifact upload has no bucket in this container; keep traces local
    import concourse.bass_utils as bu
    bu.upload_artifacts = lambda tmpdir: str(tmpdir)


def kernel(**inputs):
    from concourse.bass_utils import run_bass_kernel_spmd
    import os

    x = np.ascontiguousarray(np.asarray(inputs["x"], dtype=np.float32))
    edge_index = np.asarray(inputs["edge_index"], dtype=np.int32)
    cfg = Cfg(x.shape[0], 8)
    T_list, per_core = _preprocess(cfg, x, edge_index)
    nc = _get_program(cfg, T_list)

    wnames = []
    for li in (1, 2):
        wnames += [f"Wl{li}", f"bl{li}", f"Wr{li}", f"br{li}", f"att{li}",
                   f"bias{li}", f"gng{li}", f"gnb{li}", f"gna{li}"]
    wnames += ["W1", "b1"]
    base = {}
    for n in wnames:
        a = np.ascontiguousarray(np.asarray(inputs[n], dtype=np.float32))
        if n.startswith(("bl", "br", "bias", "gng", "gnb", "gna", "b1")):
            a = a.reshape(-1)
        base[n] = a
    in_maps = [{**base, **pc} for pc in per_core]

    trace = bool(int(os.environ.get("GAT_TRACE", "0")))
    if trace:
        _install_axon_ntff_shim()
    r = run_bass_kernel_spmd(nc, in_maps, core_ids=list(range(cfg.NC)), trace=trace)
    kernel.last_results = r
    if trace and r.exec_time_ns is not None:
        print(f"HW exec time: {r.exec_time_ns} ns")
        if r.instructions_and_trace is not None:
            print(f"trace: {r.instructions_and_trace[1]}")
        print(f"profile_json: {r.profile_json}")
        kernel.last_exec_ns = r.exec_time_ns
    out = np.concatenate([r.results[k]["out"] for k in range(cfg.NC)], axis=0)
    return out



# revision 2
# speedup vs baseline: 1.0347x; 1.0347x over previous
"""Trainium2 Bass kernel v2 for 2-layer GATv2 + GraphNorm node classifier.

Key differences vs v1 baseline (5.52ms):
  - bf16 tables + bf16 matmuls (PE fp32 streams at 4 cyc/col, bf16 at 1).
  - Per-tile gpsimd indirect_dma_start (994ns fixed each) replaced by batched
    dma_gather over groups of 4 dst-blocks: edge-major xl rows, feat-major
    (transpose) xl rows, and feat-major xr rows per edge.  int16 gather
    indices force splitting the 50176-row xl table into two 25088-row halves
    (edges are grouped by src-half on the host).
  - m = xl[s]+xr[d] built by one DVE add of two gathered feat-major tiles;
    leaky folded into pre-scaled attention weights (0.2*att @ m + 0.8*att @
    relu(m)); exp/scores batched 4 tiles per PSUM bank.
  - Biases folded out of the gather tables: score-side (bl+br) into the xr
    table, payload-side bl into the post-softmax bias (sum(alpha)=1).
"""

import numpy as np
import ml_dtypes

import concourse.bacc as bacc
import concourse.bass as bass
import concourse.mybir as mybir
import concourse.tile as tile
from concourse.masks import make_identity

F32 = mybir.dt.float32
BF16 = mybir.dt.bfloat16
I16 = mybir.dt.int16
I32 = mybir.dt.int32
AF = mybir.ActivationFunctionType
OP = mybir.AluOpType
bfloat16 = ml_dtypes.bfloat16

P = 128
GSZ = 3  # dst blocks per gather group


class Cfg:
    def __init__(self, n_nodes, n_cores=8):
        assert n_nodes % n_cores == 0
        self.N = n_nodes
        self.NC = n_cores
        self.NPC = n_nodes // n_cores          # 6250
        self.BLOCKS = (self.NPC + P - 1) // P  # 49
        self.NPADC = self.BLOCKS * P           # 6272
        self.NPAD_ALL = self.NC * self.NPADC   # 50176
        self.HALF = self.NPAD_ALL // 2         # 25088 (= 4 cores' rows)
        self.DIN = 128
        self.HC = 128
        self.C = 64
        self.NCLS = 4
        self.EPS = 1e-5
        self.NT = self.NC * self.BLOCKS        # 392 table tiles
        self.LAST = self.NPC - (self.BLOCKS - 1) * P  # 106


def _layout(cfg, T):
    """Group/slot layout from tile counts T [BLOCKS, 2]."""
    BLOCKS = cfg.BLOCKS
    NG = (BLOCKS + GSZ - 1) // GSZ
    base = np.zeros((BLOCKS, 2), np.int64)   # global slot offset per (block, half)
    groups = []
    tile0 = 0
    for g in range(NG):
        bs = list(range(g * GSZ, min((g + 1) * GSZ, BLOCKS)))
        off = tile0
        binfo = {}
        for h in (0, 1):
            for b in bs:
                base[b, h] = off * P
                binfo[(b, h)] = off - tile0
                off += int(T[b, h])
        groups.append(dict(
            blocks=bs, tile0=tile0, TG=off - tile0,
            S0t=int(sum(T[b, 0] for b in bs)), binfo=binfo))
        tile0 = off
    return groups, base, tile0  # tile0 == total tiles


def _preprocess(cfg, x, edge_index):
    N, NC, NPC, BLOCKS, NPADC = cfg.N, cfg.NC, cfg.NPC, cfg.BLOCKS, cfg.NPADC
    src = np.concatenate([edge_index[0].astype(np.int64), np.arange(N, dtype=np.int64)])
    dst = np.concatenate([edge_index[1].astype(np.int64), np.arange(N, dtype=np.int64)])

    core = dst // NPC
    dloc = dst - core * NPC
    blk = dloc // P
    dstl = dloc - blk * P
    srcr = (src // NPC) * NPADC + (src % NPC)
    half = (srcr >= cfg.HALF).astype(np.int64)
    srcl = srcr - half * cfg.HALF

    cnt = np.zeros((NC, BLOCKS, 2), np.int64)
    np.add.at(cnt, (core, blk, half), 1)
    T = np.maximum(1, (cnt.max(axis=0) + P - 1) // P)  # [BLOCKS, 2] shared

    groups, base, TOT_TILES = _layout(cfg, T)
    TOT_SLOTS = TOT_TILES * P

    key = core * (TOT_SLOTS + P) + base[blk, half]
    order = np.argsort(key, kind="stable")
    key_s = key[order]
    pos = np.arange(len(key_s)) - np.searchsorted(key_s, key_s, side="left")
    slot = base[blk[order], half[order]] + pos
    core_s = core[order]

    esrc = np.zeros((NC, TOT_SLOTS), np.int16)
    edloc = np.zeros((NC, TOT_SLOTS), np.int16)
    edstl = np.full((NC, TOT_SLOTS), -1.0, np.float32)
    esrc[core_s, slot] = srcl[order].astype(np.int16)
    edloc[core_s, slot] = dloc[order].astype(np.int16)
    edstl[core_s, slot] = dstl[order].astype(np.float32)

    # transposed padded x, bf16
    xT = np.zeros((cfg.DIN, cfg.NPAD_ALL), np.float32)
    xsrc = np.ascontiguousarray(x.T)
    for k in range(NC):
        xT[:, k * NPADC: k * NPADC + NPC] = xsrc[:, k * NPC:(k + 1) * NPC]
    xT = xT.astype(bfloat16)

    per_core = []
    for k in range(NC):
        eb = edstl[k].astype(bfloat16)
        per_core.append({
            "xT": np.ascontiguousarray(xT),
            "xTl": np.ascontiguousarray(xT[:, k * NPADC:(k + 1) * NPADC]),
            "esrc16": np.ascontiguousarray(np.tile(esrc[k].reshape(-1, 16).T, (8, 1))),
            "edstlT": np.ascontiguousarray(eb.reshape(TOT_TILES, P).T),
            "edstl_rep": np.ascontiguousarray(np.broadcast_to(eb[None, :], (P, TOT_SLOTS))),
        })
    return T, per_core


def _host_weights(cfg, inputs):
    f32 = np.float32
    base = {}
    for li in (1, 2):
        att = np.asarray(inputs[f"att{li}"], f32)          # [2, 64]
        for nm, sc in (("attm02", 0.2), ("attm08", 0.8)):
            m = np.zeros((128, 2), f32)
            m[0:64, 0] = sc * att[0]
            m[64:128, 1] = sc * att[1]
            base[f"{nm}_{li}"] = m.astype(bfloat16)
        for nm in ("gng", "gnb", "gna"):
            base[f"{nm}{li}"] = np.asarray(inputs[f"{nm}{li}"], f32).reshape(-1)
    def attm_full(li):
        att = np.asarray(inputs[f"att{li}"], f32)
        m = np.zeros((128, 2), f32)
        m[0:64, 0] = att[0]
        m[64:128, 1] = att[1]
        return m
    bl1 = np.asarray(inputs["bl1"], f32).reshape(-1)
    br1 = np.asarray(inputs["br1"], f32).reshape(-1)
    bias1 = np.asarray(inputs["bias1"], f32).reshape(-1)
    base["Wl1b"] = np.asarray(inputs["Wl1"], f32).astype(bfloat16)
    base["Wr1b"] = np.asarray(inputs["Wr1"], f32).astype(bfloat16)
    base["brep1"] = np.tile((bl1 + br1)[None, :], (P, 1)).astype(f32)
    base["comb1"] = np.tile((bias1 + 0.5 * (bl1[0:64] + bl1[64:128]))[None, :], (P, 1)).astype(f32)
    a02_1 = 0.2 * attm_full(1)
    a02_2 = 0.2 * attm_full(2)
    base["Wl02_1"] = (np.asarray(inputs["Wl1"], f32) @ a02_1).astype(bfloat16)
    base["Wr02_1"] = (np.asarray(inputs["Wr1"], f32) @ a02_1).astype(bfloat16)
    base["blbr02_1"] = np.tile(((bl1 + br1) @ a02_1)[None, :], (P, 1)).astype(f32)
    base["Wl02_2r"] = (np.asarray(inputs["Wl2"], f32) @ a02_2).astype(f32)
    base["Wr02_2r"] = (np.asarray(inputs["Wr2"], f32) @ a02_2).astype(f32)
    base["Wl2"] = np.asarray(inputs["Wl2"], f32)
    base["Wr2"] = np.asarray(inputs["Wr2"], f32)
    base["bl2"] = np.asarray(inputs["bl2"], f32).reshape(-1)
    base["br2"] = np.asarray(inputs["br2"], f32).reshape(-1)
    base["bias2"] = np.asarray(inputs["bias2"], f32).reshape(-1)
    base["W1"] = np.asarray(inputs["W1"], f32)
    base["b1"] = np.asarray(inputs["b1"], f32).reshape(-1)
    return base


def _build(cfg, T):
    NC, BLOCKS, NPADC = cfg.NC, cfg.BLOCKS, cfg.NPADC
    NPC, HC, C, NCLS, HALF = cfg.NPC, cfg.HC, cfg.C, cfg.NCLS, cfg.HALF
    LAST = cfg.LAST
    groups, base_slots, TOT_TILES = _layout(cfg, T)
    TGmax = max(G["TG"] for G in groups)
    rg = [list(range(NC))]

    nc = bacc.Bacc("TRN2", target_bir_lowering=False, debug=False,
                   enable_asserts=False, num_devices=NC)

    # ---------------- IO ----------------
    xT_d = nc.dram_tensor("xT", [128, cfg.NPAD_ALL], BF16, kind="ExternalInput")
    xTl_d = nc.dram_tensor("xTl", [128, NPADC], BF16, kind="ExternalInput")
    esrc_d = nc.dram_tensor("esrc16", [128, TOT_TILES * 8], I16, kind="ExternalInput")
    edstl_d = nc.dram_tensor("edstlT", [128, TOT_TILES], BF16, kind="ExternalInput")
    edstlrep_d = nc.dram_tensor("edstl_rep", [128, TOT_TILES * 128], BF16, kind="ExternalInput")
    w = {}
    for nm, shape, dt in [
        ("attm02_1", [128, 2], BF16), ("attm08_1", [128, 2], BF16),
        ("attm02_2", [128, 2], BF16), ("attm08_2", [128, 2], BF16),
        ("Wl1b", [128, HC], BF16), ("Wr1b", [128, HC], BF16),
        ("brep1", [P, HC], F32), ("comb1", [P, C], F32),
        ("Wl02_1", [128, 2], BF16), ("Wr02_1", [128, 2], BF16),
        ("blbr02_1", [P, 2], F32),
        ("Wl02_2r", [C, 2], F32), ("Wr02_2r", [C, 2], F32),
        ("Wl2", [C, HC], F32), ("Wr2", [C, HC], F32),
        ("bl2", [HC], F32), ("br2", [HC], F32), ("bias2", [C], F32),
        ("W1", [C, NCLS], F32), ("b1", [NCLS], F32),
        ("gng1", [C], F32), ("gnb1", [C], F32), ("gna1", [C], F32),
        ("gng2", [C], F32), ("gnb2", [C], F32), ("gna2", [C], F32),
    ]:
        w[nm] = nc.dram_tensor(nm, shape, dt, kind="ExternalInput")
    out_d = nc.dram_tensor("out", [NPC, NCLS], F32, kind="ExternalOutput")

    # ---------------- internal DRAM ----------------
    ROWW = 256  # table row: [xl(128) | a02(2) | pad]
    xl_h = {}
    for li in (1, 2):
        for h in (0, 1):
            xl_h[(li, h)] = nc.dram_tensor(f"xl{li}_h{h}", [HALF, ROWW], BF16, kind="Internal")
    h1T_dr = nc.dram_tensor("h1T_dr", [C, NPADC], BF16, kind="Internal")
    h1T_ag = nc.dram_tensor("h1T_ag", [C * NC, NPADC], BF16, kind="Internal", addr_space="Shared")
    st_l = [nc.dram_tensor(f"st{li}_l", [C, 2], F32, kind="Internal") for li in (1, 2)]
    st_g = [nc.dram_tensor(f"st{li}_g", [C, 2], F32, kind="Internal", addr_space="Shared") for li in (1, 2)]

    with tile.TileContext(nc) as tc:
        import contextlib
        ctx = contextlib.ExitStack()
        with ctx:
            con = ctx.enter_context(tc.tile_pool(name="con", bufs=1))
            res = ctx.enter_context(tc.tile_pool(name="res", bufs=1))
            tb = ctx.enter_context(tc.tile_pool(name="tb", bufs=4))
            gat = ctx.enter_context(tc.tile_pool(name="gat", bufs=2))
            idxp = ctx.enter_context(tc.tile_pool(name="idxp", bufs=2))
            qp = ctx.enter_context(tc.tile_pool(name="qp", bufs=3))
            payp = ctx.enter_context(tc.tile_pool(name="payp", bufs=3))
            ohp = ctx.enter_context(tc.tile_pool(name="ohp", bufs=4))
            dr = ctx.enter_context(tc.tile_pool(name="dr", bufs=2))
            ps_tab = ctx.enter_context(tc.tile_pool(name="ps_tab", bufs=1, space="PSUM"))
            ps_mpe = ctx.enter_context(tc.tile_pool(name="ps_mpe", bufs=2, space="PSUM"))
            ps_pet = ctx.enter_context(tc.tile_pool(name="ps_pet", bufs=2, space="PSUM"))
            ps_acc = ctx.enter_context(tc.tile_pool(name="ps_acc", bufs=2, space="PSUM"))
            ps_st = ctx.enter_context(tc.tile_pool(name="ps_st", bufs=1, space="PSUM"))

            # ---------------- constants ----------------
            ident = con.tile([P, P], F32)
            make_identity(nc, ident[:])
            iota_i = con.tile([P, P], I32)
            nc.gpsimd.iota(iota_i[:], pattern=[[1, P]], base=0, channel_multiplier=0)
            iota_b = con.tile([P, P], BF16)
            nc.vector.tensor_copy(iota_b[:], iota_i[:])
            iota_pi = con.tile([P, 1], I32)
            nc.gpsimd.iota(iota_pi[:], pattern=[[0, 1]], base=0, channel_multiplier=1)
            iota_pc = con.tile([P, 1], BF16)
            nc.vector.tensor_copy(iota_pc[:], iota_pi[:])
            ident_bf128 = con.tile([P, P], BF16)
            nc.vector.tensor_copy(ident_bf128[:], ident[:])
            ones_row = con.tile([1, P], F32)
            nc.vector.memset(ones_row[:], 1.0)
            mask_col = con.tile([P, 1], F32)
            nc.vector.memset(mask_col[:], 1.0)
            if LAST < P:
                nc.gpsimd.affine_select(
                    out=mask_col[:], in_=mask_col[:], compare_op=OP.is_ge,
                    fill=0.0, base=LAST - 1, channel_multiplier=-1, pattern=[[0, 1]])

            def load_sb(d, shape, dt, tag):
                t = con.tile(shape, dt, tag=tag)
                nc.sync.dma_start(out=t[:], in_=d[:])
                return t

            def load_row(d, n, tag):
                t = con.tile([1, n], F32, tag=tag)
                nc.sync.dma_start(out=t[:], in_=d[None, :])
                return t

            def load_col(d, n, tag):
                t = con.tile([n, 1], F32, tag=tag)
                nc.sync.dma_start(out=t[:], in_=d[:, None])
                return t

            def replicate_row(row_t, n, tag, dt=F32):
                pr = ps_tab.tile([128, 512], F32, space="PSUM", tag="ps_tab")
                nc.tensor.matmul(pr[:, 0:n], lhsT=ones_row[:], rhs=row_t[:], start=True, stop=True)
                t = con.tile([P, n], dt, tag=tag)
                nc.scalar.copy(t[:], pr[:, 0:n])
                return t

            attm = {(li, s): load_sb(w[f"attm{s}_{li}"], [128, 2], BF16, f"attm{s}_{li}")
                    for li in (1, 2) for s in ("02", "08")}
            Wl1_sb = load_sb(w["Wl1b"], [128, HC], BF16, "Wl1")
            Wr1_sb = load_sb(w["Wr1b"], [128, HC], BF16, "Wr1")
            brep1_sb = load_sb(w["brep1"], [P, HC], F32, "brep1")
            comb1_sb = load_sb(w["comb1"], [P, C], F32, "comb1")
            Wl02_1sb = load_sb(w["Wl02_1"], [128, 2], BF16, "Wl02_1")
            Wr02_1sb = load_sb(w["Wr02_1"], [128, 2], BF16, "Wr02_1")
            blbr02_1sb = load_sb(w["blbr02_1"], [P, 2], F32, "blbr02_1")
            Wl02_2r_sb = load_sb(w["Wl02_2r"], [C, 2], F32, "Wl02_2r")
            Wr02_2r_sb = load_sb(w["Wr02_2r"], [C, 2], F32, "Wr02_2r")
            Wl2_sb = load_sb(w["Wl2"], [C, HC], F32, "Wl2")
            Wr2_sb = load_sb(w["Wr2"], [C, HC], F32, "Wr2")
            W1_sb = load_sb(w["W1"], [C, NCLS], F32, "W1")
            b1_row = load_row(w["b1"], NCLS, "b1r")
            bl2_row = load_row(w["bl2"], HC, "bl2r")
            br2_row = load_row(w["br2"], HC, "br2r")
            bias2_row = load_row(w["bias2"], C, "bias2r")

            edstl_sb = con.tile([128, TOT_TILES], BF16, tag="edstl")
            nc.sync.dma_start(out=edstl_sb[:], in_=edstl_d[:])

            h1T_res = res.tile([C, NPADC], BF16, tag="h1T")
            h2T_res = res.tile([C, NPADC], BF16, tag="h2T")
            xr_res = res.tile([P, BLOCKS, HC], BF16, tag="xr_res")
            a02r_res = res.tile([P, BLOCKS, 2], BF16, tag="a02r_res")

            # ---------------- table builders ----------------
            def build_xl_table(li, lhs_loader, rhs_sb, w02_sb):
                """widened xl table rows [xl(128)|a02(2)|pad] -> xl_h[(li, 0/1)]."""
                NT = cfg.NT
                for j0 in range(0, NT, 2):
                    jn = min(2, NT - j0)
                    ps = ps_tab.tile([128, 512], F32, space="PSUM", tag="ps_tab")
                    for k in range(jn):
                        lhs = lhs_loader(j0 + k)
                        nc.tensor.matmul(ps[:, k * 256:k * 256 + 128], lhsT=lhs,
                                         rhs=rhs_sb[:], start=True, stop=True)
                        nc.tensor.matmul(ps[:, k * 256 + 128:k * 256 + 130], lhsT=lhs,
                                         rhs=w02_sb[:], start=True, stop=True)
                    t4 = tb.tile([128, 512], BF16, tag="t4")
                    for k in range(jn):
                        if (j0 // 2) % 2 == 0:
                            nc.scalar.copy(t4[:, k * 256:k * 256 + 130],
                                           ps[:, k * 256:k * 256 + 130])
                        else:
                            nc.vector.tensor_copy(t4[:, k * 256:k * 256 + 130],
                                                  ps[:, k * 256:k * 256 + 130])
                    for k in range(jn):
                        j = j0 + k
                        half, row = (0, j * 128) if j * 128 < HALF else (1, j * 128 - HALF)
                        nc.sync.dma_start(out=xl_h[(li, half)][row:row + 128, 0:130],
                                          in_=t4[:, k * 256:k * 256 + 130])

            def build_xr_res(li, lhs_loader, rhs_sb, brep, w02_sb, blbr02):
                for b in range(BLOCKS):
                    ps = ps_tab.tile([128, 512], F32, space="PSUM", tag="ps_tab")
                    lhs = lhs_loader(b)
                    nc.tensor.matmul(ps[:, 0:128], lhsT=lhs, rhs=rhs_sb[:], start=True, stop=True)
                    nc.tensor.matmul(ps[:, 128:130], lhsT=lhs, rhs=w02_sb[:], start=True, stop=True)
                    nc.vector.tensor_add(xr_res[:, b, :], ps[:, 0:128], brep[:])
                    nc.vector.tensor_add(a02r_res[:, b, :], ps[:, 128:130], blbr02[:])

            def l1_lhs(j):
                t = tb.tile([128, 128], BF16, tag="xt")
                nc.sync.dma_start(out=t[:], in_=xT_d[:, j * 128:(j + 1) * 128])
                return t[:]

            def l1_lhs_local(b):
                t = tb.tile([128, 128], BF16, tag="xt")
                nc.sync.dma_start(out=t[:], in_=xTl_d[:, b * 128:(b + 1) * 128])
                return t[:]

            # ---------------- edge phase ----------------
            def edge_phase(li, hT_res, comb_rep):
                pstats = ps_st.tile([C, C + 1], F32, space="PSUM", tag="ps_stats")
                for G in groups:
                    TG, tile0, S0t = G["TG"], G["tile0"], G["S0t"]
                    S1t = TG - S0t
                    xls = gat.tile([128, TGmax, 256], BF16, tag="xls")
                    drep = gat.tile([128, TGmax * 128], BF16, tag="drep")
                    isrc = idxp.tile([128, TGmax * 8], I16, tag="isrc")
                    nc.sync.dma_start(out=isrc[:, 0:TG * 8],
                                      in_=esrc_d[:, tile0 * 8:(tile0 + TG) * 8])
                    nc.sync.dma_start(out=drep[:, 0:TG * 128],
                                      in_=edstlrep_d[:, tile0 * 128:(tile0 + TG) * 128])
                    CH = 7  # <=896 idx per dma_gather (HW SWDGE ring cap)
                    for h, toff, ntile in ((0, 0, S0t), (1, S0t, S1t)):
                        if ntile == 0:
                            continue
                        tbl = xl_h[(li, h)]
                        for c0 in range(0, ntile, CH):
                            cn = min(CH, ntile - c0)
                            n = cn * 128
                            a, bnd = toff + c0, toff + c0 + cn
                            nc.gpsimd.dma_gather(
                                out_ap=xls[:, a:bnd, :], in_ap=tbl[:],
                                idxs_ap=isrc[:, a * 8:bnd * 8],
                                num_idxs=n, num_idxs_reg=n, elem_size=256)

                    for b in G["blocks"]:
                        runs = [(G["binfo"][(b, 0)], int(T[b, 0])),
                                (G["binfo"][(b, 1)], int(T[b, 1]))]
                        total_tiles = int(T[b, 0] + T[b, 1])
                        acc = ps_acc.tile([P, HC + 2], F32, space="PSUM", tag="ps_acc")
                        done = 0
                        for roff, rcnt in runs:
                            for q0 in range(0, rcnt, 2):
                                q = min(2, rcnt - q0)
                                t0 = roff + q0
                                # mp bank: m_k at [:, k*128:(k+1)*128], pe at [0:2, 256:512]
                                mp = ps_mpe.tile([128, 512], F32, space="PSUM", tag="ps_mpe")
                                ohTs = []
                                for k in range(q):
                                    t = t0 + k
                                    ohT = ohp.tile([P, P], BF16, tag="ohT")
                                    nc.vector.tensor_tensor(
                                        out=ohT[:], in0=iota_pc[:, 0:1].to_broadcast([P, P]),
                                        in1=drep[:, t * 128:(t + 1) * 128],
                                        op=OP.is_equal)
                                    ohTs.append(ohT)
                                    nc.tensor.matmul(mp[:, k * 128:(k + 1) * 128],
                                                     lhsT=xls[:, t, 0:128], rhs=ident_bf128[:],
                                                     start=True, stop=False)
                                    nc.tensor.matmul(mp[:, k * 128:(k + 1) * 128],
                                                     lhsT=xr_res[:, b, :], rhs=ohT[:],
                                                     start=False, stop=True)
                                r2 = qp.tile([128, 256], BF16, tag="r2")
                                nc.scalar.activation(r2[:, 0:q * 128], mp[:, 0:q * 128], AF.Relu)
                                for k in range(q):
                                    t = t0 + k
                                    reg = mp[0:2, 256 + k * 128:256 + (k + 1) * 128]
                                    nc.tensor.matmul(reg, lhsT=xls[:, t, 128:130],
                                                     rhs=ident_bf128[:], start=True, stop=False)
                                    nc.tensor.matmul(reg, lhsT=a02r_res[:, b, :], rhs=ohTs[k][:],
                                                     start=False, stop=False)
                                    nc.tensor.matmul(reg, lhsT=attm[(li, "08")][:],
                                                     rhs=r2[:, k * 128:(k + 1) * 128],
                                                     start=False, stop=True)
                                ee = qp.tile([2, 256], F32, tag="ee")
                                nc.scalar.activation(ee[:, 0:q * 128], mp[0:2, 256:256 + q * 128], AF.Exp)
                                pet = ps_pet.tile([128, 136], F32, space="PSUM", tag="ps_pet")
                                pay4 = payp.tile([128, 2, 130], BF16, tag="pay4")
                                for k in range(q):
                                    nc.tensor.transpose(pet[:, 2 * k:2 * k + 2],
                                                        ee[:, k * 128:(k + 1) * 128],
                                                        ident[0:2, 0:2])
                                eesb = qp.tile([128, 4], F32, tag="eesb")
                                nc.scalar.copy(eesb[:, 0:2 * q], pet[:, 0:2 * q])
                                nc.vector.tensor_copy(pay4[:, 0:q, 128:130], eesb[:, 0:2 * q])
                                for k in range(q):
                                    t = t0 + k
                                    oh = ohp.tile([P, P], BF16, tag="oh")
                                    nc.vector.tensor_tensor(
                                        out=oh[:], in0=iota_b[:],
                                        in1=edstl_sb[:, tile0 + t:tile0 + t + 1].to_broadcast([P, P]),
                                        op=OP.is_equal)
                                    nc.scalar.activation(pay4[:, k, 0:C], xls[:, t, 0:C],
                                                         AF.Copy, scale=eesb[:, 2 * k:2 * k + 1])
                                    nc.vector.tensor_scalar_mul(
                                        pay4[:, k, C:HC], xls[:, t, C:HC], eesb[:, 2 * k + 1:2 * k + 2])
                                    nc.tensor.matmul(acc[:], lhsT=oh[:], rhs=pay4[:, k, :],
                                                     start=(done == 0),
                                                     stop=(done == total_tiles - 1))
                                    done += 1
                        # ---- drain block b ----
                        d2 = dr.tile([P, 2], F32, tag="d2")
                        nc.scalar.activation(d2[:], acc[:, HC:HC + 2], AF.Copy,
                                             bias=1e-20, scale=2.0)
                        rec = dr.tile([P, 2], F32, tag="rec")
                        nc.vector.reciprocal(rec[:], d2[:])
                        hs = dr.tile([P, C + 1], F32, tag="hs")
                        nc.vector.tensor_scalar_mul(hs[:, 0:C], acc[:, 0:C], rec[:, 0:1])
                        t1 = dr.tile([P, C], F32, tag="t1")
                        nc.vector.tensor_scalar_mul(t1[:], acc[:, C:HC], rec[:, 1:2])
                        nc.vector.tensor_add(hs[:, 0:C], hs[:, 0:C], t1[:])
                        nc.vector.tensor_add(hs[:, 0:C], hs[:, 0:C], comb_rep[:])
                        nc.vector.memset(hs[:, C:C + 1], 1.0)
                        if b == BLOCKS - 1 and LAST < P:
                            nc.vector.tensor_scalar_mul(hs[:], hs[:], mask_col[:, 0:1])
                        nc.tensor.matmul(pstats[:], lhsT=hs[:, 0:C], rhs=hs[:],
                                         start=(b == 0), stop=(b == BLOCKS - 1))
                        phtt = ps_pet.tile([128, 136], F32, space="PSUM", tag="ps_pet")
                        nc.tensor.transpose(phtt[0:C, 8:136], hs[:, 0:C], ident[:])
                        nc.scalar.copy(hT_res[:, b * 128:(b + 1) * 128], phtt[0:C, 8:136])

                # ---- GraphNorm stats -> A, B ----
                li_i = li - 1
                trash = dr.tile([C, C], F32, tag="trash")
                st2 = dr.tile([C, 2], F32, tag="st2")
                nc.vector.tensor_mul(trash[:], pstats[:, 0:C], ident[0:C, 0:C])
                nc.vector.tensor_reduce(st2[:, 1:2], trash[:], axis=mybir.AxisListType.X, op=OP.add)
                nc.vector.tensor_copy(st2[:, 0:1], pstats[:, C:C + 1])
                nc.sync.dma_start(out=st_l[li_i][:], in_=st2[:])
                nc.gpsimd.collective_compute(
                    "AllReduce", OP.add, replica_groups=rg,
                    ins=[st_l[li_i][:]], outs=[st_g[li_i][:]])
                stg = dr.tile([C, 2], F32, tag="stg")
                nc.sync.dma_start(out=stg[:], in_=st_g[li_i][:])
                a_col = load_col(w[f"gna{li}"], C, f"gna{li}")
                g_col = load_col(w[f"gng{li}"], C, f"gng{li}")
                bta_col = load_col(w[f"gnb{li}"], C, f"gnb{li}")
                mean = dr.tile([C, 1], F32, tag="gn_m")
                nc.scalar.activation(mean[:], stg[:, 0:1], AF.Copy, bias=0.0, scale=1.0 / cfg.N)
                msq = dr.tile([C, 1], F32, tag="gn_m2")
                nc.scalar.square(msq[:], mean[:])
                qn = dr.tile([C, 1], F32, tag="gn_qn")
                nc.scalar.activation(qn[:], stg[:, 1:2], AF.Copy, bias=0.0, scale=1.0 / cfg.N)
                a2 = dr.tile([C, 1], F32, tag="gn_a2")
                nc.vector.tensor_mul(a2[:], a_col[:], a_col[:])
                twoa = dr.tile([C, 1], F32, tag="gn_2a")
                nc.scalar.activation(twoa[:], a_col[:], AF.Copy, bias=0.0, scale=2.0)
                coef = dr.tile([C, 1], F32, tag="gn_cf")
                nc.vector.tensor_sub(coef[:], twoa[:], a2[:])
                cm = dr.tile([C, 1], F32, tag="gn_cm")
                nc.vector.tensor_mul(cm[:], coef[:], msq[:])
                var = dr.tile([C, 1], F32, tag="gn_var")
                nc.vector.tensor_sub(var[:], qn[:], cm[:])
                vare = dr.tile([C, 1], F32, tag="gn_vare")
                nc.vector.tensor_scalar_add(vare[:], var[:], cfg.EPS)
                lnv = dr.tile([C, 1], F32, tag="gn_lnv")
                nc.scalar.activation(lnv[:], vare[:], AF.Ln)
                rs = dr.tile([C, 1], F32, tag="gn_rs")
                nc.scalar.activation(rs[:], lnv[:], AF.Exp, bias=0.0, scale=-0.5)
                A = con.tile([C, 1], F32, tag=f"gn_A{li}")
                nc.vector.tensor_mul(A[:], g_col[:], rs[:])
                t_ = dr.tile([C, 1], F32, tag="gn_t")
                nc.vector.tensor_mul(t_[:], A[:], a_col[:])
                t2_ = dr.tile([C, 1], F32, tag="gn_t2")
                nc.vector.tensor_mul(t2_[:], t_[:], mean[:])
                B = con.tile([C, 1], F32, tag=f"gn_B{li}")
                nc.vector.tensor_sub(B[:], bta_col[:], t2_[:])
                return A, B

            # ================ layer 1 ================
            build_xl_table(1, l1_lhs, Wl1_sb, Wl02_1sb)
            build_xr_res(1, l1_lhs_local, Wr1_sb, brep1_sb, Wr02_1sb, blbr02_1sb)
            A1, B1 = edge_phase(1, h1T_res, comb1_sb)

            # AllGather h1 (bf16, transposed layout)
            nc.sync.dma_start(out=h1T_dr[:], in_=h1T_res[:])
            nc.gpsimd.collective_compute(
                "AllGather", OP.bypass, replica_groups=rg,
                ins=[h1T_dr[:]], outs=[h1T_ag[:]])

            # folded layer-2 weights
            def fold_w(W_sb, b_row, A, B, tag):
                Wp = con.tile([C, HC], BF16, tag=f"W_{tag}")
                nc.vector.tensor_scalar_mul(Wp[:], W_sb[:], A[:])
                pb = ps_tab.tile([128, 512], F32, space="PSUM", tag="ps_tab")
                nc.tensor.matmul(pb[0:1, 0:HC], lhsT=B[:], rhs=W_sb[:], start=True, stop=True)
                brow = con.tile([1, HC], F32, tag=f"brow_{tag}")
                nc.vector.tensor_add(brow[:], pb[0:1, 0:HC], b_row[:])
                return Wp, brow

            Wl2p, bl2p_row = fold_w(Wl2_sb, bl2_row, A1, B1, "l2l")
            Wr2p, br2p_row = fold_w(Wr2_sb, br2_row, A1, B1, "l2r")
            xrb2_row = con.tile([1, HC], F32, tag="xrb2_row")
            nc.vector.tensor_add(xrb2_row[:], bl2p_row[:], br2p_row[:])
            brep2_sb = replicate_row(xrb2_row, HC, "brep2")
            # folded a02 mats for layer 2
            M2_2 = con.tile([C, 2], BF16, tag="M2_2")
            nc.vector.tensor_scalar_mul(M2_2[:], Wl02_2r_sb[:], A1[:])
            Wr02_2sb = con.tile([C, 2], BF16, tag="Wr02_2")
            nc.vector.tensor_scalar_mul(Wr02_2sb[:], Wr02_2r_sb[:], A1[:])
            # blbr02_2 = xrb2_row @ attm02_2, replicated to [P, 2]
            xrb2_bf = con.tile([1, HC], BF16, tag="xrb2_bf")
            nc.vector.tensor_copy(xrb2_bf[:], xrb2_row[:])
            pcol = ps_tab.tile([128, 512], F32, space="PSUM", tag="ps_tab")
            nc.tensor.matmul(pcol[:, 0:1], lhsT=xrb2_bf[:], rhs=ident_bf128[0:1, 0:1],
                             start=True, stop=True)
            xrb2c = con.tile([P, 1], BF16, tag="xrb2c")
            nc.scalar.copy(xrb2c[:], pcol[:, 0:1])
            psb2 = ps_tab.tile([128, 512], F32, space="PSUM", tag="ps_tab")
            nc.tensor.matmul(psb2[0:1, 0:2], lhsT=xrb2c[:], rhs=attm[(2, "02")][:],
                             start=True, stop=True)
            blbr02_2row = con.tile([1, 2], F32, tag="blbr02_2row")
            nc.vector.tensor_copy(blbr02_2row[:], psb2[0:1, 0:2])
            blbr02_2rep = replicate_row(blbr02_2row, 2, "blbr02_2rep")
            comb2_row = con.tile([1, C], F32, tag="comb2_row")
            nc.vector.tensor_add(comb2_row[:], bl2p_row[:, 0:C], bl2p_row[:, C:HC])
            nc.scalar.activation(comb2_row[:], comb2_row[:], AF.Copy, bias=0.0, scale=0.5)
            nc.vector.tensor_add(comb2_row[:], comb2_row[:], bias2_row[:])
            comb2_sb = replicate_row(comb2_row, C, "comb2")

            # ================ layer 2 ================
            def l2_lhs(j):
                k, b = divmod(j, BLOCKS)
                t = tb.tile([C, 128], BF16, tag="ht")
                nc.sync.dma_start(out=t[:], in_=h1T_ag[k * C:(k + 1) * C, b * 128:(b + 1) * 128])
                return t[:]

            def l2_lhs_local(b):
                return h1T_res[:, b * 128:(b + 1) * 128]

            build_xl_table(2, l2_lhs, Wl2p, M2_2)
            build_xr_res(2, l2_lhs_local, Wr2p, brep2_sb, Wr02_2sb, blbr02_2rep)
            A2, B2 = edge_phase(2, h2T_res, comb2_sb)

            # ---------------- classifier + log_softmax ----------------
            W1p = con.tile([C, NCLS], BF16, tag="W1p")
            nc.vector.tensor_scalar_mul(W1p[:], W1_sb[:], A2[:])
            pb1 = ps_tab.tile([128, 512], F32, space="PSUM", tag="ps_tab")
            nc.tensor.matmul(pb1[0:1, 0:NCLS], lhsT=B2[:], rhs=W1_sb[:], start=True, stop=True)
            b1p = con.tile([1, NCLS], F32, tag="b1p")
            nc.vector.tensor_add(b1p[:], pb1[0:1, 0:NCLS], b1_row[:])
            b1p_rep = replicate_row(b1p, NCLS, "b1p_rep")

            for b in range(BLOCKS):
                pl = ps_acc.tile([P, HC + 2], F32, space="PSUM", tag="ps_acc")
                nc.tensor.matmul(pl[:, 0:NCLS], lhsT=h2T_res[:, b * 128:(b + 1) * 128],
                                 rhs=W1p[:], start=True, stop=True)
                lg = dr.tile([P, NCLS], F32, tag="lg")
                nc.vector.tensor_add(lg[:], pl[:, 0:NCLS], b1p_rep[:])
                mx = dr.tile([P, 1], F32, tag="mx")
                nc.vector.tensor_reduce(mx[:], lg[:], axis=mybir.AxisListType.X, op=OP.max)
                lgm = dr.tile([P, NCLS], F32, tag="lgm")
                nc.vector.tensor_scalar(out=lgm[:], in0=lg[:], scalar1=mx[:, 0:1],
                                        scalar2=None, op0=OP.subtract)
                ex = dr.tile([P, NCLS], F32, tag="ex")
                nc.scalar.activation(ex[:], lgm[:], AF.Exp)
                sm = dr.tile([P, 1], F32, tag="sm")
                nc.vector.tensor_reduce(sm[:], ex[:], axis=mybir.AxisListType.X, op=OP.add)
                lns = dr.tile([P, 1], F32, tag="lns")
                nc.scalar.activation(lns[:], sm[:], AF.Ln)
                ot = dr.tile([P, NCLS], F32, tag="ot")
                nc.vector.tensor_scalar(out=ot[:], in0=lgm[:], scalar1=lns[:, 0:1],
                                        scalar2=None, op0=OP.subtract)
                rows = min(P, NPC - b * P)
                nc.sync.dma_start(out=out_d[b * P: b * P + rows, :], in_=ot[0:rows, :])

    nc.compile()
    return nc


_CACHE = {}


def _get_program(cfg, T):
    key = tuple(T.reshape(-1).tolist())
    if key not in _CACHE:
        _CACHE[key] = _build(cfg, T)
    return _CACHE[key]


def _install_axon_ntff_shim():
    import sys, types, ctypes, contextlib, glob as _glob
    try:
        import antenv.axon_hooks  # noqa
        return
    except ImportError:
        pass
    hook = None
    for so_path in (["/opt/axon/libaxon_pjrt.so"] + _glob.glob("/root/.axon_site/**/libaxon_pjrt.so", recursive=True)):
        try:
            lib = ctypes.CDLL(so_path)
        except OSError:
            continue
        if not hasattr(lib, "axon_start_nrt_profile"):
            continue
        lib.axon_start_nrt_profile.argtypes = [ctypes.POINTER(ctypes.c_int64), ctypes.c_size_t]
        lib.axon_start_nrt_profile.restype = ctypes.c_int64
        lib.axon_stop_nrt_profile.argtypes = [ctypes.c_char_p]
        lib.axon_stop_nrt_profile.restype = ctypes.c_int64

        @contextlib.contextmanager
        def _hook(output_dir, device_ids, _lib=lib):
            import jax
            jax.devices()
            if device_ids:
                ids = (ctypes.c_int64 * len(device_ids))(*device_ids)
                rc = _lib.axon_start_nrt_profile(ids, len(device_ids))
            else:
                rc = _lib.axon_start_nrt_profile(None, 0)
            if rc != 0:
                raise RuntimeError(f"axon_start_nrt_profile rc={rc}")
            try:
                yield
            finally:
                n = _lib.axon_stop_nrt_profile(str(output_dir).encode())
                print(f"ntff profile: {n} file(s) -> {output_dir}")

        hook = _hook
        break
    m = types.ModuleType("antenv.axon_hooks")
    m.get_axon_ntff_profile_hook = lambda: hook
    m.set_axon_ntff_profile_hook = lambda h: None
    sys.modules["antenv.axon_hooks"] = m
    try:
        import antenv
        antenv.axon_hooks = m
    except ImportError:
        pass
    import concourse.bass_utils as bu
    bu.upload_artifacts = lambda tmpdir: str(tmpdir)


def kernel(**inputs):
    from concourse.bass_utils import run_bass_kernel_spmd
    import os

    x = np.ascontiguousarray(np.asarray(inputs["x"], dtype=np.float32))
    edge_index = np.asarray(inputs["edge_index"], dtype=np.int32)
    cfg = Cfg(x.shape[0], 8)
    T, per_core = _preprocess(cfg, x, edge_index)
    nc = _get_program(cfg, T)

    base = _host_weights(cfg, inputs)
    in_maps = [{**base, **pc} for pc in per_core]

    trace = bool(int(os.environ.get("GAT_TRACE", "0")))
    if trace:
        _install_axon_ntff_shim()
    r = run_bass_kernel_spmd(nc, in_maps, core_ids=list(range(cfg.NC)), trace=trace)
    kernel.last_results = r
    if trace and r.exec_time_ns is not None:
        print(f"HW exec time: {r.exec_time_ns} ns")
        if r.instructions_and_trace is not None:
            print(f"trace: {r.instructions_and_trace[1]}")
        print(f"profile_json: {r.profile_json}")
        kernel.last_exec_ns = r.exec_time_ns
    out = np.concatenate([r.results[k]["out"] for k in range(cfg.NC)], axis=0)
    return out
